# revision 1
# baseline (speedup 1.0000x reference)
"""Trainium2 Bass kernel for nn_ConditionalFeedForward (MoE routed SwiGLU FFN).

Strategy (expert-parallel, routed):
  - The reference computes every expert for every token, then gathers the
    TOP_K=2 routed experts.  Only the routed (token, expert) pairs are
    needed, so on the host we bucket tokens by expert (deduplicating
    tokens that pick the same expert twice), pad each bucket to a fixed
    capacity C, and give expert e's bucket to NeuronCore e (E=8 experts,
    8 cores).
  - Each core computes  y = (silu(xg @ w1e.T) * (xg @ w3e.T)) @ w2e.T
    for its C routed tokens with fp32r (FP22-truncated, full-rate)
    matmuls, all intermediates resident in SBUF.
  - The host scatters each core's rows back into the (T, TOP_K, D) output.

Device dataflow per core (all tokens of one expert):
  phase 1:  h1T/h3T tiles [h=128, c<=512] accumulate over d in PSUM;
            silu+mul drain into a resident SBUF tensor a[H, C].
  phase 2:  y[c=128, dd<=512] accumulates over all 32 h-tiles in PSUM
            (lhsT = a-tile, rhs = w2T tile), then drains to DRAM.
"""

import numpy as np

# Environment paths for the concourse/bass stack (present on the runner
# image; PYTHONPATH normally already includes them).
import sys

for _p in ("/opt/trn_rl_repo", "/root/.axon_site/_ro/trn_rl_repo"):
    if _p not in sys.path:
        sys.path.append(_p)

T = 4096
E = 8
D = 1024
H = 4096
TOP_K = 2
P = 128  # SBUF partitions

_PROG_CACHE: dict = {}
LAST_RUN = None  # BassKernelResults of the most recent device run (for test.py)


def _build_program(C: int, Dm: int, Hm: int):
    """Emit the per-core Bass/Tile program for capacity-C routed tokens."""
    import concourse.bass as bass  # noqa: F401
    import concourse.mybir as mybir
    from concourse import bacc
    from concourse.tile import TileContext

    f32 = mybir.dt.float32
    f32r = mybir.dt.float32r
    SIG = mybir.ActivationFunctionType.Sigmoid

    KD = Dm // P          # d-tiles (contraction of phase 1)
    NH = Hm // P          # h-tiles
    CC = C // 512         # 512-wide token chunks
    CT = C // P           # 128-wide token tiles (phase-2 output partitions)
    DDH = Dm // 512       # 512-wide output-dim chunks

    nc = bacc.Bacc("TRN2", target_bir_lowering=False)

    xgt = nc.dram_tensor("xgt", [Dm, C], f32r, kind="ExternalInput")
    w1t = nc.dram_tensor("w1t", [Dm, Hm], f32r, kind="ExternalInput")
    w3t = nc.dram_tensor("w3t", [Dm, Hm], f32r, kind="ExternalInput")
    w2t = nc.dram_tensor("w2t", [Hm, Dm], f32r, kind="ExternalInput")
    y = nc.dram_tensor("y", [C, Dm], f32, kind="ExternalOutput")

    xgt_r = xgt[:].rearrange("(do di) c -> di do c", di=P)   # [128, KD, C]
    w1t_r = w1t[:].rearrange("(do di) h -> di do h", di=P)   # [128, KD, H]
    w3t_r = w3t[:].rearrange("(do di) h -> di do h", di=P)

    with TileContext(nc) as tc:
        with (
            tc.tile_pool(name="xg", bufs=1) as xg_pool,
            tc.tile_pool(name="abuf", bufs=1) as a_pool,
            tc.tile_pool(name="w13", bufs=2) as w13_pool,
            tc.tile_pool(name="w2", bufs=6) as w2_pool,
            tc.tile_pool(name="scratch", bufs=8) as scratch_pool,
        ):
            # h=0 weight tiles are the first matmul's dependency — issue
            # their DMAs before the xg stream so the queues serve them first.
            # (Starting matmuls even earlier by reordering xg d-tiles was
            # tried and regressed: arrival-paced matmuls run HAM-cold.)
            w1_sb0 = w13_pool.tile([P, KD, P], f32r, tag="w1")
            nc.sync.dma_start(out=w1_sb0, in_=w1t_r[:, :, 0:P])
            w3_sb0 = w13_pool.tile([P, KD, P], f32r, tag="w3")
            nc.sync.dma_start(out=w3_sb0, in_=w3t_r[:, :, 0:P])
            xg_sb = xg_pool.tile([P, KD, C], f32r)
            for d in range(KD):
                nc.sync.dma_start(out=xg_sb[:, d, :], in_=xgt_r[:, d, :])
            a_sb = a_pool.tile([P, NH, C], f32r)

            # ---- phase 1: a[h, c] = silu(w1.T x) * (w3.T x) ----
            with tc.tile_pool(name="ps1", bufs=8, space="PSUM") as ps1:
                for h in range(NH):
                    hs = slice(h * P, (h + 1) * P)
                    if h == 0:
                        w1_sb, w3_sb = w1_sb0, w3_sb0
                    else:
                        w1_sb = w13_pool.tile([P, KD, P], f32r, tag="w1")
                        nc.sync.dma_start(out=w1_sb, in_=w1t_r[:, :, hs])
                        w3_sb = w13_pool.tile([P, KD, P], f32r, tag="w3")
                        nc.sync.dma_start(out=w3_sb, in_=w3t_r[:, :, hs])

                    h1_ps = [
                        ps1.tile([P, 512], f32, tag="ps", name=f"h1_{h}_{i}")
                        for i in range(CC)
                    ]
                    h3_ps = [
                        ps1.tile([P, 512], f32, tag="ps", name=f"h3_{h}_{i}")
                        for i in range(CC)
                    ]
                    for d in range(KD):
                        for cc in range(CC):
                            cs = slice(cc * 512, (cc + 1) * 512)
                            nc.tensor.matmul(
                                h1_ps[cc],
                                w1_sb[:, d, :],
                                xg_sb[:, d, cs],
                                start=(d == 0),
                                stop=(d == KD - 1),
                            )
                    for d in range(KD):
                        for cc in range(CC):
                            cs = slice(cc * 512, (cc + 1) * 512)
                            nc.tensor.matmul(
                                h3_ps[cc],
                                w3_sb[:, d, :],
                                xg_sb[:, d, cs],
                                start=(d == 0),
                                stop=(d == KD - 1),
                            )
                    for cc in range(CC):
                        cs = slice(cc * 512, (cc + 1) * 512)
                        s_sb = scratch_pool.tile([P, 512], f32, tag="scratch")
                        nc.scalar.activation(s_sb, h1_ps[cc], SIG)
                        nc.vector.tensor_mul(out=s_sb, in0=s_sb, in1=h1_ps[cc])
                        nc.vector.tensor_mul(
                            out=a_sb[:, h, cs], in0=s_sb, in1=h3_ps[cc]
                        )

            # ---- phase 2: y[c, dd] = sum_h a[h, c].T @ w2T[h, dd] ----
            # groups of <=8 token tiles so PSUM holds every accumulator
            groups = []
            for ddh in range(DDH):
                for c0 in range(0, CT, 8):
                    groups.append((ddh, list(range(c0, min(c0 + 8, CT)))))
            PREFETCH_H = 3  # next-group w2 tiles issued before this group's drains

            def w2_load(ddh, h):
                dds = slice(ddh * 512, (ddh + 1) * 512)
                w2_sb = w2_pool.tile([P, 512], f32r, tag="w2", name=f"w2_{ddh}_{h}")
                nc.sync.dma_start(out=w2_sb, in_=w2t[h * P : (h + 1) * P, dds])
                return w2_sb

            with tc.tile_pool(name="ps2", bufs=8, space="PSUM") as ps2:
                prefetched: dict = {}
                for gi, (ddh, cts) in enumerate(groups):
                    dds = slice(ddh * 512, (ddh + 1) * 512)
                    y_ps = {
                        c: ps2.tile([P, 512], f32, tag="y", name=f"y_{ddh}_{c}")
                        for c in cts
                    }
                    for h in range(NH):
                        w2_sb = prefetched.pop((gi, h), None)
                        if w2_sb is None:
                            w2_sb = w2_load(ddh, h)
                        for c in cts:
                            nc.tensor.matmul(
                                y_ps[c],
                                a_sb[:, h, c * P : (c + 1) * P],
                                w2_sb[:],
                                start=(h == 0),
                                stop=(h == NH - 1),
                            )
                    # keep the sync ring free for the next group's weights:
                    # issue those loads first, and push the drains through the
                    # scalar HWDGE ring instead of sync
                    if gi + 1 < len(groups):
                        nddh, _ = groups[gi + 1]
                        for h in range(PREFETCH_H):
                            prefetched[(gi + 1, h)] = w2_load(nddh, h)
                    for c in cts:
                        y_sb = scratch_pool.tile([P, 512], f32, tag="scratch", name=f"ysb_{gi}_{c}")
                        nc.vector.tensor_copy(out=y_sb, in_=y_ps[c])
                        nc.scalar.dma_start(
                            out=y[c * P : (c + 1) * P, dds], in_=y_sb
                        )
    nc.compile()  # bacc passes: split multi-waits, alloc regs, fuse nops
    return nc


def _get_program(C: int, Dm: int, Hm: int):
    key = (C, Dm, Hm)
    if key not in _PROG_CACHE:
        _PROG_CACHE[key] = _build_program(C, Dm, Hm)
    return _PROG_CACHE[key]


def kernel(x, expert_indices, w1, w2, w3):
    global LAST_RUN
    from concourse.bass_utils import run_bass_kernel_spmd

    x = np.ascontiguousarray(np.asarray(x, dtype=np.float32))
    idx = np.asarray(expert_indices)
    w1 = np.asarray(w1, dtype=np.float32)
    w2 = np.asarray(w2, dtype=np.float32)
    w3 = np.asarray(w3, dtype=np.float32)

    Tn, Kn = idx.shape
    Dm = x.shape[1]
    En, Hm, _ = w1.shape
    assert En == 8, f"kernel is hardcoded for 8 experts on 8 cores, got {En}"
    idx64 = idx.astype(np.int64)

    # Host routing: unique token list per expert.
    toks = [np.nonzero((idx64 == e).any(axis=1))[0] for e in range(En)]
    maxc = max(len(t) for t in toks)
    C = max(1024, -(-maxc // 512) * 512)

    nc = _get_program(C, Dm, Hm)

    in_maps = []
    for e in range(En):
        te = toks[e]
        xg = np.zeros((C, Dm), np.float32)
        xg[: len(te)] = x[te]
        in_maps.append(
            {
                "xgt": np.ascontiguousarray(xg.T),
                "w1t": np.ascontiguousarray(w1[e].T),
                "w3t": np.ascontiguousarray(w3[e].T),
                "w2t": np.ascontiguousarray(w2[e].T),
            }
        )

    LAST_RUN = run_bass_kernel_spmd(nc, in_maps, list(range(En)))
    res = LAST_RUN.results

    out = np.empty((Tn, Kn, Dm), np.float32)
    for e in range(En):
        t_arr, k_arr = np.nonzero(idx64 == e)
        pos = np.searchsorted(toks[e], t_arr)
        out[t_arr, k_arr] = res[e]["y"][pos]
    return out



# revision 2
# speedup vs baseline: 1.0985x; 1.0985x over previous
"""Trainium2 Bass kernel for nn_ConditionalFeedForward (MoE routed SwiGLU FFN).

Strategy (expert-parallel, routed):
  - Only the routed (token, expert) pairs are needed: on the host we bucket
    tokens by expert (deduplicating tokens that pick the same expert twice),
    pad each bucket to a fixed capacity C (= max bucket rounded up to 8),
    and give expert e's bucket to NeuronCore e (E=8 experts, 8 cores).
  - All operands are cast to bf16 on the host (free): halves DMA traffic,
    enables the compiler's fast-weight-load path, and keeps rel-err ~4e-3
    (threshold 2e-2).  Accumulation stays fp32 in PSUM.
  - Each core computes  yT = w2e @ (silu(w1e xg) * (w3e xg))  for its C
    routed tokens, everything SBUF-resident.
  - The host transposes/casts yT back and scatters rows into (T, TOP_K, D).

Device dataflow per core:
  warmup:   5 matmuls on a memset tile keep the PE HAM busy while the first
            DMAs land, so real matmuls start at the 2.4 GHz warm clock.
  phase 1:  h1/h3 tiles [h=128, c-chunk<=512] accumulate over d in PSUM;
            Silu+mul drain into a resident SBUF tensor a[H, C] (bf16).
            w2 is prefetched into SBUF (one h-tile per loop iteration) so
            phase 2 has no input DMA at all.
  phase 2:  yT[dd=128, c-chunk] accumulates over all 32 h-tiles in PSUM
            (lhsT = resident w2 tile [128,128] -> few, contiguous weight
            loads), in 4 dd-groups of 2 so drains overlap the next group's
            matmuls and the final drain is small.
"""

import numpy as np
import ml_dtypes

import sys

for _p in ("/opt/trn_rl_repo", "/root/.axon_site/_ro/trn_rl_repo"):
    if _p not in sys.path:
        sys.path.append(_p)

T = 4096
E = 8
D = 1024
H = 4096
TOP_K = 2
P = 128  # SBUF partitions

BF16 = ml_dtypes.bfloat16

_PROG_CACHE: dict = {}
LAST_RUN = None  # BassKernelResults of the most recent device run (for test.py)


def _chunks(C: int, width: int = 512):
    """Split [0, C) into (start, width) chunks each <= width."""
    out = []
    c0 = 0
    while c0 < C:
        w = min(width, C - c0)
        out.append((c0, w))
        c0 += w
    return out


def _build_program(C: int, Dm: int, Hm: int):
    """Emit the per-core Bass/Tile program for capacity-C routed tokens."""
    import concourse.bass as bass  # noqa: F401
    import concourse.mybir as mybir
    from concourse import bacc
    from concourse.tile import TileContext

    f32 = mybir.dt.float32
    bf16 = mybir.dt.bfloat16
    SILU = mybir.ActivationFunctionType.Silu

    KD = Dm // P          # d-tiles (contraction of phase 1)
    NH = Hm // P          # h-tiles
    CCH = _chunks(C)      # token chunks (<=512 wide)
    ND = Dm // P          # output-dim 128-tiles (phase 2)

    nc = bacc.Bacc("TRN2", target_bir_lowering=False)

    xgt = nc.dram_tensor("xgt", [Dm, C], bf16, kind="ExternalInput")
    w1t = nc.dram_tensor("w1t", [Dm, Hm], bf16, kind="ExternalInput")
    w3t = nc.dram_tensor("w3t", [Dm, Hm], bf16, kind="ExternalInput")
    w2t = nc.dram_tensor("w2t", [Hm, Dm], bf16, kind="ExternalInput")
    yt = nc.dram_tensor("yt", [Dm, C], bf16, kind="ExternalOutput")

    xgt_r = xgt[:].rearrange("(do di) c -> di do c", di=P)   # [128, KD, C]
    w1t_r = w1t[:].rearrange("(do di) h -> di do h", di=P)   # [128, KD, H]
    w3t_r = w3t[:].rearrange("(do di) h -> di do h", di=P)

    with TileContext(nc) as tc:
        with (
            tc.tile_pool(name="xg", bufs=1) as xg_pool,
            tc.tile_pool(name="abuf", bufs=1) as a_pool,
            tc.tile_pool(name="w2r", bufs=1) as w2_pool,
            tc.tile_pool(name="w13", bufs=2) as w13_pool,
            tc.tile_pool(name="scratch", bufs=4) as scratch_pool,
            tc.tile_pool(name="ydrain", bufs=4) as y_pool,
            tc.tile_pool(name="warm", bufs=1) as warm_pool,
        ):
            # ---- HAM warm-up: PE busy from program start (no DMA deps) ----
            wu_sb = warm_pool.tile([P, 512], bf16)
            nc.vector.memset(wu_sb, 0.0)
            with tc.tile_pool(name="psw", bufs=1, space="PSUM") as psw:
                wu_ps = psw.tile([P, 512], f32)
                for _ in range(5):
                    nc.tensor.matmul(wu_ps, wu_sb[:, 0:P], wu_sb, start=True, stop=True)

            # ---- head DMAs, priority order, spread over both HWDGE rings ----
            # sync ring: weights; scalar ring: xg then the w2 prefetch stream.
            w1_sb0 = w13_pool.tile([P, KD, P], bf16, tag="w1")
            nc.sync.dma_start(out=w1_sb0[:, 0:1, :], in_=w1t_r[:, 0:1, 0:P])
            xg_sb = xg_pool.tile([P, KD, C], bf16)
            nc.scalar.dma_start(out=xg_sb[:, 0, :], in_=xgt_r[:, 0, :])
            nc.sync.dma_start(out=w1_sb0[:, 1:KD, :], in_=w1t_r[:, 1:KD, 0:P])
            w3_sb0 = w13_pool.tile([P, KD, P], bf16, tag="w3")
            nc.sync.dma_start(out=w3_sb0, in_=w3t_r[:, :, 0:P])
            for d in range(1, KD):
                nc.scalar.dma_start(out=xg_sb[:, d, :], in_=xgt_r[:, d, :])

            a_sb = a_pool.tile([P, NH, C], bf16)
            w2_sb = w2_pool.tile([P, NH, Dm], bf16)

            # ---- phase 1: a[h, c] = silu(w1.T x) * (w3.T x) ----
            with tc.tile_pool(name="ps1", bufs=8, space="PSUM") as ps1:
                for h in range(NH):
                    hs = slice(h * P, (h + 1) * P)
                    if h == 0:
                        w1_sb, w3_sb = w1_sb0, w3_sb0
                    else:
                        w1_sb = w13_pool.tile([P, KD, P], bf16, tag="w1")
                        nc.sync.dma_start(out=w1_sb, in_=w1t_r[:, :, hs])
                        w3_sb = w13_pool.tile([P, KD, P], bf16, tag="w3")
                        nc.sync.dma_start(out=w3_sb, in_=w3t_r[:, :, hs])
                    # w2 prefetch for phase 2 (scalar ring, off critical path)
                    nc.scalar.dma_start(
                        out=w2_sb[:, h, :], in_=w2t[h * P : (h + 1) * P, :]
                    )

                    h1_ps = [
                        ps1.tile([P, cw], f32, tag="ps", name=f"h1_{h}_{i}")
                        for i, (c0, cw) in enumerate(CCH)
                    ]
                    h3_ps = [
                        ps1.tile([P, cw], f32, tag="ps", name=f"h3_{h}_{i}")
                        for i, (c0, cw) in enumerate(CCH)
                    ]
                    for d in range(KD):
                        for i, (c0, cw) in enumerate(CCH):
                            nc.tensor.matmul(
                                h1_ps[i],
                                w1_sb[:, d, :],
                                xg_sb[:, d, c0 : c0 + cw],
                                start=(d == 0),
                                stop=(d == KD - 1),
                            )
                    for d in range(KD):
                        for i, (c0, cw) in enumerate(CCH):
                            nc.tensor.matmul(
                                h3_ps[i],
                                w3_sb[:, d, :],
                                xg_sb[:, d, c0 : c0 + cw],
                                start=(d == 0),
                                stop=(d == KD - 1),
                            )
                    for i, (c0, cw) in enumerate(CCH):
                        s_sb = scratch_pool.tile([P, 512], f32, tag="scratch")
                        nc.scalar.activation(s_sb[:, 0:cw], h1_ps[i], SILU)
                        nc.vector.tensor_mul(
                            out=a_sb[:, h, c0 : c0 + cw],
                            in0=s_sb[:, 0:cw],
                            in1=h3_ps[i],
                        )

            # ---- phase 2: yT[dd, c] = sum_h w2T[h, dd].T @ a[h, c] ----
            # dd-groups of 2 so PSUM holds two groups -> drains overlap the
            # next group's matmuls and the final drain is small.
            DDG = 2
            with tc.tile_pool(name="ps2", bufs=8, space="PSUM") as ps2:
                for dg in range(0, ND, DDG):
                    dds = list(range(dg, min(dg + DDG, ND)))
                    y_ps = {
                        (dd, i): ps2.tile([P, cw], f32, tag="y", name=f"y_{dd}_{i}")
                        for dd in dds
                        for i, (c0, cw) in enumerate(CCH)
                    }
                    for h in range(NH):
                        for dd in dds:
                            w2_w = w2_sb[:, h, dd * P : (dd + 1) * P]
                            for i, (c0, cw) in enumerate(CCH):
                                nc.tensor.matmul(
                                    y_ps[(dd, i)],
                                    w2_w,
                                    a_sb[:, h, c0 : c0 + cw],
                                    start=(h == 0),
                                    stop=(h == NH - 1),
                                )
                    for dd in dds:
                        for i, (c0, cw) in enumerate(CCH):
                            y_sb = y_pool.tile(
                                [P, 512], bf16, tag="y", name=f"ysb_{dd}_{i}"
                            )
                            nc.vector.tensor_copy(
                                out=y_sb[:, 0:cw], in_=y_ps[(dd, i)]
                            )
                            nc.sync.dma_start(
                                out=yt[dd * P : (dd + 1) * P, c0 : c0 + cw],
                                in_=y_sb[:, 0:cw],
                            )
    nc.compile()  # bacc passes: split multi-waits, alloc regs, fuse nops
    return nc


def _get_program(C: int, Dm: int, Hm: int):
    key = (C, Dm, Hm)
    if key not in _PROG_CACHE:
        _PROG_CACHE[key] = _build_program(C, Dm, Hm)
    return _PROG_CACHE[key]


def kernel(x, expert_indices, w1, w2, w3):
    global LAST_RUN
    from concourse.bass_utils import run_bass_kernel_spmd

    x = np.ascontiguousarray(np.asarray(x, dtype=np.float32))
    idx = np.asarray(expert_indices)
    w1 = np.asarray(w1, dtype=np.float32)
    w2 = np.asarray(w2, dtype=np.float32)
    w3 = np.asarray(w3, dtype=np.float32)

    Tn, Kn = idx.shape
    Dm = x.shape[1]
    En, Hm, _ = w1.shape
    assert En == 8, f"kernel is hardcoded for 8 experts on 8 cores, got {En}"
    idx64 = idx.astype(np.int64)

    # Host routing: unique token list per expert.
    toks = [np.nonzero((idx64 == e).any(axis=1))[0] for e in range(En)]
    maxc = max(len(t) for t in toks)
    C = max(512, -(-maxc // 8) * 8)

    nc = _get_program(C, Dm, Hm)

    in_maps = []
    for e in range(En):
        te = toks[e]
        xg = np.zeros((C, Dm), np.float32)
        xg[: len(te)] = x[te]
        in_maps.append(
            {
                "xgt": np.ascontiguousarray(xg.T.astype(BF16)),
                "w1t": np.ascontiguousarray(w1[e].T.astype(BF16)),
                "w3t": np.ascontiguousarray(w3[e].T.astype(BF16)),
                "w2t": np.ascontiguousarray(w2[e].T.astype(BF16)),
            }
        )

    LAST_RUN = run_bass_kernel_spmd(nc, in_maps, list(range(En)))
    res = LAST_RUN.results

    out = np.empty((Tn, Kn, Dm), np.float32)
    for e in range(En):
        ye = np.asarray(res[e]["yt"]).astype(np.float32).T  # [C, Dm]
        t_arr, k_arr = np.nonzero(idx64 == e)
        pos = np.searchsorted(toks[e], t_arr)
        out[t_arr, k_arr] = ye[pos]
    return out


# revision 3
# speedup vs baseline: 1.1052x; 1.0061x over previous
"""Trainium2 Bass kernel for nn_ConditionalFeedForward (MoE routed SwiGLU FFN).

Strategy (expert-parallel, routed):
  - Only the routed (token, expert) pairs are needed: on the host we bucket
    tokens by expert (deduplicating tokens that pick the same expert twice),
    pad each bucket to a fixed capacity C (= max bucket rounded up to 8),
    and give expert e's bucket to NeuronCore e (E=8 experts, 8 cores).
  - All operands are cast to bf16 on the host (free): halves DMA traffic,
    enables the compiler's fast-weight-load path, and keeps rel-err ~4e-3
    (threshold 2e-2).  Accumulation stays fp32 in PSUM.
  - Each core computes  yT = w2e @ (silu(w1e xg) * (w3e xg))  for its C
    routed tokens, everything SBUF-resident.
  - The host transposes/casts yT back and scatters rows into (T, TOP_K, D).

Device dataflow per core:
  warmup:   7 matmuls on a memset tile keep the PE HAM busy while the first
            DMAs land, so real matmuls run at the 2.4 GHz warm clock early.
  phase 1:  h1/h3 tiles [h=128, c-chunk<=512] accumulate over d in PSUM;
            Silu+mul drain into resident SBUF tensors a_lo/a_hi (bf16).
            The first h-tile's weights arrive via dedicated contiguous
            tensors (w1h0c/w3h0c) so the head DMAs run at line rate.
            w2 is prefetched into SBUF (two h-tiles per loop iteration,
            starting at h=1 to keep the head DMA window clear), so phase 2
            has no input DMA at all.
  phase 2:  yT[dd=128, c-chunk] accumulates over all 32 h-tiles in PSUM
            (lhsT = resident w2 tile [128,128] -> few, contiguous weight
            loads), in dd-groups of [2,2,2,1,1] so drains overlap the next
            group's matmuls and the final drain is small; the last drains
            split across the vector/scalar engines and both HWDGE rings.
"""

import numpy as np
import ml_dtypes

import sys

for _p in ("/opt/trn_rl_repo", "/root/.axon_site/_ro/trn_rl_repo"):
    if _p not in sys.path:
        sys.path.append(_p)

T = 4096
E = 8
D = 1024
H = 4096
TOP_K = 2
P = 128  # SBUF partitions

BF16 = ml_dtypes.bfloat16

_PROG_CACHE: dict = {}
LAST_RUN = None  # BassKernelResults of the most recent device run (for test.py)


def _chunks(C: int, width: int = 512):
    """Split [0, C) into (start, width) chunks each <= width."""
    out = []
    c0 = 0
    while c0 < C:
        w = min(width, C - c0)
        out.append((c0, w))
        c0 += w
    return out


def _build_program(C: int, Dm: int, Hm: int):
    """Emit the per-core Bass/Tile program for capacity-C routed tokens."""
    import concourse.bass as bass  # noqa: F401
    import concourse.mybir as mybir
    from concourse import bacc
    from concourse.tile import TileContext

    f32 = mybir.dt.float32
    bf16 = mybir.dt.bfloat16
    SILU = mybir.ActivationFunctionType.Silu
    COPY = mybir.ActivationFunctionType.Copy

    KD = Dm // P          # d-tiles (contraction of phase 1)
    NH = Hm // P          # h-tiles
    NHL = NH // 2         # h-tiles in a_lo
    CCH = _chunks(C)      # token chunks (<=512 wide)
    ND = Dm // P          # output-dim 128-tiles (phase 2)

    nc = bacc.Bacc("TRN2", target_bir_lowering=False)

    xgt = nc.dram_tensor("xgt", [Dm, C], bf16, kind="ExternalInput")
    w1t = nc.dram_tensor("w1t", [Dm, Hm], bf16, kind="ExternalInput")
    w3t = nc.dram_tensor("w3t", [Dm, Hm], bf16, kind="ExternalInput")
    w2t = nc.dram_tensor("w2t", [Hm, Dm], bf16, kind="ExternalInput")
    # first h-tile of w1/w3 pre-tiled on the host: [di, do*128+hh], contiguous
    w1h0c = nc.dram_tensor("w1h0c", [P, KD * P], bf16, kind="ExternalInput")
    w3h0c = nc.dram_tensor("w3h0c", [P, KD * P], bf16, kind="ExternalInput")
    yt = nc.dram_tensor("yt", [Dm, C], bf16, kind="ExternalOutput")

    xgt_r = xgt[:].rearrange("(do di) c -> di do c", di=P)   # [128, KD, C]
    w1t_r = w1t[:].rearrange("(do di) h -> di do h", di=P)   # [128, KD, H]
    w3t_r = w3t[:].rearrange("(do di) h -> di do h", di=P)

    with TileContext(nc) as tc:
        with (
            tc.tile_pool(name="xg", bufs=1) as xg_pool,
            tc.tile_pool(name="abuf", bufs=1) as a_pool,
            tc.tile_pool(name="w2r", bufs=1) as w2_pool,
            tc.tile_pool(name="w13", bufs=2) as w13_pool,
            tc.tile_pool(name="scratch", bufs=4) as scratch_pool,
            tc.tile_pool(name="ydrain", bufs=4) as y_pool,
            tc.tile_pool(name="warm", bufs=1) as warm_pool,
        ):
            # ---- HAM warm-up: PE busy from program start (no DMA deps) ----
            wu_sb = warm_pool.tile([P, 512], bf16)
            nc.gpsimd.memset(wu_sb, 0.0)
            with tc.tile_pool(name="psw", bufs=1, space="PSUM") as psw:
                wu_ps = psw.tile([P, 512], f32)
                for _ in range(7):
                    nc.tensor.matmul(wu_ps, wu_sb[:, 0:P], wu_sb, start=True, stop=True)

            # ---- head DMAs, priority order, spread over both HWDGE rings ----
            # sync ring: weights; scalar ring: xg (w2 prefetch joins at h>=1).
            w1_sb0 = w13_pool.tile([P, KD, P], bf16, tag="w1h0")
            nc.sync.dma_start(out=w1_sb0, in_=w1h0c[:].rearrange("p (do h) -> p do h", do=KD))
            xg_sb = xg_pool.tile([P, KD, C], bf16)
            nc.scalar.dma_start(out=xg_sb[:, 0, :], in_=xgt_r[:, 0, :])
            w3_sb0 = w13_pool.tile([P, KD, P], bf16, tag="w3h0")
            nc.sync.dma_start(out=w3_sb0, in_=w3h0c[:].rearrange("p (do h) -> p do h", do=KD))
            for d in range(1, KD):
                nc.scalar.dma_start(out=xg_sb[:, d, :], in_=xgt_r[:, d, :])

            a_lo = a_pool.tile([P, NHL, C], bf16, tag="alo")
            a_hi = a_pool.tile([P, NH - NHL, C], bf16, tag="ahi")

            def a_slice(h, c0, cw):
                if h < NHL:
                    return a_lo[:, h, c0 : c0 + cw]
                return a_hi[:, h - NHL, c0 : c0 + cw]

            w2_sb = w2_pool.tile([P, NH, Dm], bf16)

            # ---- phase 1: a[h, c] = silu(w1.T x) * (w3.T x) ----
            with tc.tile_pool(name="ps1", bufs=8, space="PSUM") as ps1:
                for h in range(NH):
                    hs = slice(h * P, (h + 1) * P)
                    if h == 0:
                        w1_sb, w3_sb = w1_sb0, w3_sb0
                    else:
                        w1_sb = w13_pool.tile([P, KD, P], bf16, tag="w1")
                        nc.sync.dma_start(out=w1_sb, in_=w1t_r[:, :, hs])
                        w3_sb = w13_pool.tile([P, KD, P], bf16, tag="w3")
                        nc.sync.dma_start(out=w3_sb, in_=w3t_r[:, :, hs])
                        # w2 prefetch for phase 2 (scalar ring, after the
                        # head xg stream; two h-tiles per iteration)
                        for h2 in (2 * (h - 1), 2 * (h - 1) + 1):
                            if h2 < NH:
                                nc.scalar.dma_start(
                                    out=w2_sb[:, h2, :],
                                    in_=w2t[h2 * P : (h2 + 1) * P, :],
                                )

                    h1_ps = [
                        ps1.tile([P, cw], f32, tag="ps", name=f"h1_{h}_{i}")
                        for i, (c0, cw) in enumerate(CCH)
                    ]
                    h3_ps = [
                        ps1.tile([P, cw], f32, tag="ps", name=f"h3_{h}_{i}")
                        for i, (c0, cw) in enumerate(CCH)
                    ]
                    for d in range(KD):
                        for i, (c0, cw) in enumerate(CCH):
                            nc.tensor.matmul(
                                h1_ps[i],
                                w1_sb[:, d, :],
                                xg_sb[:, d, c0 : c0 + cw],
                                start=(d == 0),
                                stop=(d == KD - 1),
                            )
                    for d in range(KD):
                        for i, (c0, cw) in enumerate(CCH):
                            nc.tensor.matmul(
                                h3_ps[i],
                                w3_sb[:, d, :],
                                xg_sb[:, d, c0 : c0 + cw],
                                start=(d == 0),
                                stop=(d == KD - 1),
                            )
                    for i, (c0, cw) in enumerate(CCH):
                        s_sb = scratch_pool.tile([P, 512], f32, tag="scratch")
                        nc.scalar.activation(s_sb[:, 0:cw], h1_ps[i], SILU)
                        nc.vector.tensor_mul(
                            out=a_slice(h, c0, cw),
                            in0=s_sb[:, 0:cw],
                            in1=h3_ps[i],
                        )

            # ---- phase 2: yT[dd, c] = sum_h w2T[h, dd].T @ a[h, c] ----
            # dd-groups sized [2,2,2,1,1]: drains overlap the next group's
            # matmuls and the final drain is small.
            ddgs = []
            nd = 0
            for g in (2, 2, 2, 1, 1):
                ddgs.append(list(range(nd, min(nd + g, ND))))
                nd += g
            assert nd == ND, (nd, ND)

            with tc.tile_pool(name="ps2", bufs=8, space="PSUM") as ps2:
                for gi, dds in enumerate(ddgs):
                    y_ps = {
                        (dd, i): ps2.tile([P, cw], f32, tag="y", name=f"y_{dd}_{i}")
                        for dd in dds
                        for i, (c0, cw) in enumerate(CCH)
                    }
                    for h in range(NH):
                        for dd in dds:
                            w2_w = w2_sb[:, h, dd * P : (dd + 1) * P]
                            for i, (c0, cw) in enumerate(CCH):
                                nc.tensor.matmul(
                                    y_ps[(dd, i)],
                                    w2_w,
                                    a_slice(h, c0, cw),
                                    start=(h == 0),
                                    stop=(h == NH - 1),
                                )
                    last = gi == len(ddgs) - 1
                    for dd in dds:
                        for i, (c0, cw) in enumerate(CCH):
                            y_sb = y_pool.tile(
                                [P, 512], bf16, tag="y", name=f"ysb_{dd}_{i}"
                            )
                            if last and i % 2 == 1:
                                # parallelize the final drain across engines
                                nc.scalar.activation(
                                    y_sb[:, 0:cw], y_ps[(dd, i)], COPY
                                )
                                nc.scalar.dma_start(
                                    out=yt[dd * P : (dd + 1) * P, c0 : c0 + cw],
                                    in_=y_sb[:, 0:cw],
                                )
                            else:
                                nc.vector.tensor_copy(
                                    out=y_sb[:, 0:cw], in_=y_ps[(dd, i)]
                                )
                                nc.sync.dma_start(
                                    out=yt[dd * P : (dd + 1) * P, c0 : c0 + cw],
                                    in_=y_sb[:, 0:cw],
                                )
    nc.compile()  # bacc passes: split multi-waits, alloc regs, fuse nops
    return nc


def _get_program(C: int, Dm: int, Hm: int):
    key = (C, Dm, Hm)
    if key not in _PROG_CACHE:
        _PROG_CACHE[key] = _build_program(C, Dm, Hm)
    return _PROG_CACHE[key]


def kernel(x, expert_indices, w1, w2, w3):
    global LAST_RUN
    from concourse.bass_utils import run_bass_kernel_spmd

    x = np.ascontiguousarray(np.asarray(x, dtype=np.float32))
    idx = np.asarray(expert_indices)
    w1 = np.asarray(w1, dtype=np.float32)
    w2 = np.asarray(w2, dtype=np.float32)
    w3 = np.asarray(w3, dtype=np.float32)

    Tn, Kn = idx.shape
    Dm = x.shape[1]
    En, Hm, _ = w1.shape
    assert En == 8, f"kernel is hardcoded for 8 experts on 8 cores, got {En}"
    idx64 = idx.astype(np.int64)
    KD = Dm // P

    # Host routing: unique token list per expert.
    toks = [np.nonzero((idx64 == e).any(axis=1))[0] for e in range(En)]
    maxc = max(len(t) for t in toks)
    C = max(512, -(-maxc // 8) * 8)

    nc = _get_program(C, Dm, Hm)

    in_maps = []
    for e in range(En):
        te = toks[e]
        xg = np.zeros((C, Dm), np.float32)
        xg[: len(te)] = x[te]
        w1te = np.ascontiguousarray(w1[e].T.astype(BF16))  # [Dm, Hm]
        w3te = np.ascontiguousarray(w3[e].T.astype(BF16))
        # first h-tile pre-tiled: [di, do, hh] contiguous per partition
        w1h0c = np.ascontiguousarray(
            w1te[:, 0:P].reshape(KD, P, P).transpose(1, 0, 2).reshape(P, KD * P)
        )
        w3h0c = np.ascontiguousarray(
            w3te[:, 0:P].reshape(KD, P, P).transpose(1, 0, 2).reshape(P, KD * P)
        )
        in_maps.append(
            {
                "xgt": np.ascontiguousarray(xg.T.astype(BF16)),
                "w1t": w1te,
                "w3t": w3te,
                "w2t": np.ascontiguousarray(w2[e].T.astype(BF16)),
                "w1h0c": w1h0c,
                "w3h0c": w3h0c,
            }
        )

    LAST_RUN = run_bass_kernel_spmd(nc, in_maps, list(range(En)))
    res = LAST_RUN.results

    out = np.empty((Tn, Kn, Dm), np.float32)
    for e in range(En):
        ye = np.asarray(res[e]["yt"]).astype(np.float32).T  # [C, Dm]
        t_arr, k_arr = np.nonzero(idx64 == e)
        pos = np.searchsorted(toks[e], t_arr)
        out[t_arr, k_arr] = ye[pos]
    return out


# revision 7
# speedup vs baseline: 1.1100x; 1.0043x over previous
"""Trainium2 Bass kernel for nn_ConditionalFeedForward (MoE routed SwiGLU FFN).

Strategy (expert-parallel, routed):
  - Only the routed (token, expert) pairs are needed: on the host we bucket
    tokens by expert (deduplicating tokens that pick the same expert twice),
    pad each bucket to a fixed capacity C (= max bucket rounded up to 8),
    and give expert e's bucket to NeuronCore e (E=8 experts, 8 cores).
  - All operands are cast to bf16 on the host (free): halves DMA traffic,
    enables the compiler's fast-weight-load path, and keeps rel-err ~4e-3
    (threshold 2e-2).  Accumulation stays fp32 in PSUM.
  - Each core computes  yT = w2e @ (silu(w1e xg) * (w3e xg))  for its C
    routed tokens, everything SBUF-resident.
  - The host transposes/casts yT back and scatters rows into (T, TOP_K, D).

Device dataflow per core:
  warmup:   7 matmuls on a memset tile keep the PE HAM busy while the first
            DMAs land, so real matmuls run at the 2.4 GHz warm clock early.
  phase 1:  h1/h3 tiles [h=128, c-chunk<=512] accumulate over d in PSUM;
            Silu+mul drain into resident SBUF tensors a_lo/a_hi (bf16).
            The first h-tile's weights arrive via dedicated contiguous
            tensors (w1h0c/w3h0c) so the head DMAs run at line rate.
            w2 is prefetched into SBUF (two h-tiles per loop iteration,
            starting at h=1 to keep the head DMA window clear), so phase 2
            has no input DMA at all.
  phase 2:  yT[dd=128, c-chunk] accumulates over all 32 h-tiles in PSUM
            (lhsT = resident w2 tile [128,128] -> few, contiguous weight
            loads), in dd-groups of [2,2,2,1,1] so drains overlap the next
            group's matmuls and the final drain is small; the last drains
            split across the vector/scalar engines and both HWDGE rings.
"""

import numpy as np
import ml_dtypes

import sys

for _p in ("/opt/trn_rl_repo", "/root/.axon_site/_ro/trn_rl_repo"):
    if _p not in sys.path:
        sys.path.append(_p)

T = 4096
E = 8
D = 1024
H = 4096
TOP_K = 2
P = 128  # SBUF partitions

BF16 = ml_dtypes.bfloat16

_PROG_CACHE: dict = {}
LAST_RUN = None  # BassKernelResults of the most recent device run (for test.py)


def _chunks(C: int, width: int = 512):
    """Split [0, C) into (start, width) chunks each <= width."""
    out = []
    c0 = 0
    while c0 < C:
        w = min(width, C - c0)
        out.append((c0, w))
        c0 += w
    return out


def _build_program(C: int, Dm: int, Hm: int):
    """Emit the per-core Bass/Tile program for capacity-C routed tokens."""
    import concourse.bass as bass  # noqa: F401
    import concourse.mybir as mybir
    from concourse import bacc
    from concourse.tile import TileContext

    f32 = mybir.dt.float32
    bf16 = mybir.dt.bfloat16
    SILU = mybir.ActivationFunctionType.Silu
    COPY = mybir.ActivationFunctionType.Copy

    KD = Dm // P          # d-tiles (contraction of phase 1)
    NH = Hm // P          # h-tiles
    NHL = NH // 2         # h-tiles in a_lo
    CCH = _chunks(C)      # token chunks (<=512 wide)
    ND = Dm // P          # output-dim 128-tiles (phase 2)

    nc = bacc.Bacc("TRN2", target_bir_lowering=False)

    xgt = nc.dram_tensor("xgt", [Dm, C], bf16, kind="ExternalInput")
    w1t = nc.dram_tensor("w1t", [Dm, Hm], bf16, kind="ExternalInput")
    w3t = nc.dram_tensor("w3t", [Dm, Hm], bf16, kind="ExternalInput")
    w2t = nc.dram_tensor("w2t", [Hm, Dm], bf16, kind="ExternalInput")
    # first h-tile of w1/w3 pre-tiled on the host: [di, do*128+hh], contiguous
    w1h0c = nc.dram_tensor("w1h0c", [P, KD * P], bf16, kind="ExternalInput")
    w3h0c = nc.dram_tensor("w3h0c", [P, KD * P], bf16, kind="ExternalInput")
    yt = nc.dram_tensor("yt", [Dm, C], bf16, kind="ExternalOutput")

    xgt_r = xgt[:].rearrange("(do di) c -> di do c", di=P)   # [128, KD, C]
    w1t_r = w1t[:].rearrange("(do di) h -> di do h", di=P)   # [128, KD, H]
    w3t_r = w3t[:].rearrange("(do di) h -> di do h", di=P)

    with TileContext(nc) as tc:
        with (
            tc.tile_pool(name="xg", bufs=1) as xg_pool,
            tc.tile_pool(name="abuf", bufs=1) as a_pool,
            tc.tile_pool(name="w2r", bufs=1) as w2_pool,
            tc.tile_pool(name="w13", bufs=2) as w13_pool,
            tc.tile_pool(name="scratch", bufs=4) as scratch_pool,
            tc.tile_pool(name="ydrain", bufs=4) as y_pool,
            tc.tile_pool(name="warm", bufs=1) as warm_pool,
        ):
            # ---- HAM warm-up: PE busy from program start (no DMA deps) ----
            wu_sb = warm_pool.tile([P, 512], bf16)
            nc.gpsimd.memset(wu_sb, 0.0)

            # ---- head DMAs, priority order, spread over both HWDGE rings ----
            # sync ring: weights; scalar ring: xg (w2 prefetch joins at h>=1).
            w1_sb0 = w13_pool.tile([P, KD, P], bf16, tag="w1h0")
            nc.sync.dma_start(out=w1_sb0, in_=w1h0c[:].rearrange("p (do h) -> p do h", do=KD))
            xg_sb = xg_pool.tile([P, KD, C], bf16)
            nc.scalar.dma_start(out=xg_sb[:, 0, :], in_=xgt_r[:, 0, :])
            w3_sb0 = w13_pool.tile([P, KD, P], bf16, tag="w3h0")
            nc.sync.dma_start(out=w3_sb0, in_=w3h0c[:].rearrange("p (do h) -> p do h", do=KD))
            for d in range(1, KD):
                nc.scalar.dma_start(out=xg_sb[:, d, :], in_=xgt_r[:, d, :])

            a_lo = a_pool.tile([P, NHL, C], bf16, tag="alo")
            a_hi = a_pool.tile([P, NH - NHL, C], bf16, tag="ahi")

            def a_slice(h, c0, cw):
                if h < NHL:
                    return a_lo[:, h, c0 : c0 + cw]
                return a_hi[:, h - NHL, c0 : c0 + cw]

            w2_sb = w2_pool.tile([P, NH, Dm], bf16)

            # ---- one PSUM pool for warmup + both phases: no pool-close
            # ---- barrier between phase 1's last drains and phase 2
            with tc.tile_pool(name="ps", bufs=8, space="PSUM") as ps1:
                ps2 = ps1
                wu_ps = ps1.tile([P, 512], f32, tag="ps", name="warm")
                for _ in range(2):
                    nc.tensor.matmul(wu_ps, wu_sb[:, 0:P], wu_sb, start=True, stop=True)

                # ---- phase 1: a[h, c] = silu(w1.T x) * (w3.T x) ----
                for h in range(NH):
                    hs = slice(h * P, (h + 1) * P)
                    if h == 0:
                        w1_sb, w3_sb = w1_sb0, w3_sb0
                    else:
                        w1_sb = w13_pool.tile([P, KD, P], bf16, tag="w1")
                        nc.sync.dma_start(out=w1_sb, in_=w1t_r[:, :, hs])
                        w3_sb = w13_pool.tile([P, KD, P], bf16, tag="w3")
                        nc.sync.dma_start(out=w3_sb, in_=w3t_r[:, :, hs])
                        # w2 prefetch for phase 2 (scalar ring, after the
                        # head xg stream; two h-tiles per iteration)
                        for h2 in (2 * (h - 1), 2 * (h - 1) + 1):
                            if h2 < NH:
                                nc.scalar.dma_start(
                                    out=w2_sb[:, h2, :],
                                    in_=w2t[h2 * P : (h2 + 1) * P, :],
                                )

                    h1_ps = [
                        ps1.tile([P, cw], f32, tag="ps", name=f"h1_{h}_{i}")
                        for i, (c0, cw) in enumerate(CCH)
                    ]
                    h3_ps = [
                        ps1.tile([P, cw], f32, tag="ps", name=f"h3_{h}_{i}")
                        for i, (c0, cw) in enumerate(CCH)
                    ]
                    # h==0 is paced by xg arrival: interleave w1/w3 per
                    # d-tile so each arriving xg slice unlocks 4 matmuls.
                    if h == 0:
                        wave = [(w1_sb, h1_ps), (w3_sb, h3_ps)]
                        for d in range(KD):
                            for w_sb, ps in wave:
                                for i, (c0, cw) in enumerate(CCH):
                                    nc.tensor.matmul(
                                        ps[i],
                                        w_sb[:, d, :],
                                        xg_sb[:, d, c0 : c0 + cw],
                                        start=(d == 0),
                                        stop=(d == KD - 1),
                                    )
                    else:
                        for d in range(KD):
                            for i, (c0, cw) in enumerate(CCH):
                                nc.tensor.matmul(
                                    h1_ps[i],
                                    w1_sb[:, d, :],
                                    xg_sb[:, d, c0 : c0 + cw],
                                    start=(d == 0),
                                    stop=(d == KD - 1),
                                )
                        for d in range(KD):
                            for i, (c0, cw) in enumerate(CCH):
                                nc.tensor.matmul(
                                    h3_ps[i],
                                    w3_sb[:, d, :],
                                    xg_sb[:, d, c0 : c0 + cw],
                                    start=(d == 0),
                                    stop=(d == KD - 1),
                                )
                    for i, (c0, cw) in enumerate(CCH):
                        s_sb = scratch_pool.tile([P, 512], f32, tag="scratch")
                        nc.scalar.activation(s_sb[:, 0:cw], h1_ps[i], SILU)
                        nc.vector.tensor_mul(
                            out=a_slice(h, c0, cw),
                            in0=s_sb[:, 0:cw],
                            in1=h3_ps[i],
                        )

                # ---- phase 2: yT[dd, c] = sum_h w2T[h, dd].T @ a[h, c] ----
                # dd-groups sized [2,2,2,1,1]: drains overlap the next
                # group's matmuls and the final drain is small.
                ddgs = []
                nd = 0
                for g in (2, 2, 2, 1, 1):
                    ddgs.append(list(range(nd, min(nd + g, ND))))
                    nd += g
                assert nd == ND, (nd, ND)

                for gi, dds in enumerate(ddgs):
                    y_ps = {
                        (dd, i): ps2.tile([P, cw], f32, tag="ps", name=f"y_{dd}_{i}")
                        for dd in dds
                        for i, (c0, cw) in enumerate(CCH)
                    }
                    for h in range(NH):
                        for dd in dds:
                            w2_w = w2_sb[:, h, dd * P : (dd + 1) * P]
                            for i, (c0, cw) in enumerate(CCH):
                                nc.tensor.matmul(
                                    y_ps[(dd, i)],
                                    w2_w,
                                    a_slice(h, c0, cw),
                                    start=(h == 0),
                                    stop=(h == NH - 1),
                                )
                    last = gi == len(ddgs) - 1
                    for dd in dds:
                        for i, (c0, cw) in enumerate(CCH):
                            y_sb = y_pool.tile(
                                [P, 512], bf16, tag="y", name=f"ysb_{dd}_{i}"
                            )
                            if last and i % 2 == 1:
                                # parallelize the final drain across engines
                                nc.scalar.activation(
                                    y_sb[:, 0:cw], y_ps[(dd, i)], COPY
                                )
                                nc.scalar.dma_start(
                                    out=yt[dd * P : (dd + 1) * P, c0 : c0 + cw],
                                    in_=y_sb[:, 0:cw],
                                )
                            else:
                                nc.vector.tensor_copy(
                                    out=y_sb[:, 0:cw], in_=y_ps[(dd, i)]
                                )
                                nc.sync.dma_start(
                                    out=yt[dd * P : (dd + 1) * P, c0 : c0 + cw],
                                    in_=y_sb[:, 0:cw],
                                )
    nc.compile()  # bacc passes: split multi-waits, alloc regs, fuse nops
    return nc


def _get_program(C: int, Dm: int, Hm: int):
    key = (C, Dm, Hm)
    if key not in _PROG_CACHE:
        _PROG_CACHE[key] = _build_program(C, Dm, Hm)
    return _PROG_CACHE[key]


def kernel(x, expert_indices, w1, w2, w3):
    global LAST_RUN
    from concourse.bass_utils import run_bass_kernel_spmd

    x = np.ascontiguousarray(np.asarray(x, dtype=np.float32))
    idx = np.asarray(expert_indices)
    w1 = np.asarray(w1, dtype=np.float32)
    w2 = np.asarray(w2, dtype=np.float32)
    w3 = np.asarray(w3, dtype=np.float32)

    Tn, Kn = idx.shape
    Dm = x.shape[1]
    En, Hm, _ = w1.shape
    assert En == 8, f"kernel is hardcoded for 8 experts on 8 cores, got {En}"
    idx64 = idx.astype(np.int64)
    KD = Dm // P

    # Host routing: unique token list per expert.
    toks = [np.nonzero((idx64 == e).any(axis=1))[0] for e in range(En)]
    maxc = max(len(t) for t in toks)
    C = max(512, -(-maxc // 8) * 8)

    nc = _get_program(C, Dm, Hm)

    in_maps = []
    for e in range(En):
        te = toks[e]
        xg = np.zeros((C, Dm), np.float32)
        xg[: len(te)] = x[te]
        w1te = np.ascontiguousarray(w1[e].T.astype(BF16))  # [Dm, Hm]
        w3te = np.ascontiguousarray(w3[e].T.astype(BF16))
        # first h-tile pre-tiled: [di, do, hh] contiguous per partition
        w1h0c = np.ascontiguousarray(
            w1te[:, 0:P].reshape(KD, P, P).transpose(1, 0, 2).reshape(P, KD * P)
        )
        w3h0c = np.ascontiguousarray(
            w3te[:, 0:P].reshape(KD, P, P).transpose(1, 0, 2).reshape(P, KD * P)
        )
        in_maps.append(
            {
                "xgt": np.ascontiguousarray(xg.T.astype(BF16)),
                "w1t": w1te,
                "w3t": w3te,
                "w2t": np.ascontiguousarray(w2[e].T.astype(BF16)),
                "w1h0c": w1h0c,
                "w3h0c": w3h0c,
            }
        )

    LAST_RUN = run_bass_kernel_spmd(nc, in_maps, list(range(En)))
    res = LAST_RUN.results

    out = np.empty((Tn, Kn, Dm), np.float32)
    for e in range(En):
        ye = np.asarray(res[e]["yt"]).astype(np.float32).T  # [C, Dm]
        t_arr, k_arr = np.nonzero(idx64 == e)
        pos = np.searchsorted(toks[e], t_arr)
        out[t_arr, k_arr] = ye[pos]
    return out
